# revision 3
# baseline (speedup 1.0000x reference)
"""GQA kernel for Trainium2, 8 NeuronCores.

Sharding: (batch x kv-head) — cores 0-3 handle batch 0, cores 4-7 batch 1;
each core owns 2 KV heads (8 Q heads). Row-parallel Wo via AllGather of the
attention outputs (bf16) within each 4-core group; each core computes a
512-column slice of the final output.

B=2, S=2048, H=2048, NH=32, NKV=8, HD=64. All matmuls bf16 (f32 PSUM);
softmax statistics in f32. Causal structure exploited: fully-masked upper
tiles skipped; diagonal tiles masked multiplicatively with exp(mask*scale)
built from the mask data itself.
"""
import numpy as np

import concourse.bass as bass
import concourse.tile as tile
from concourse import mybir
from concourse.bass_utils import run_bass_kernel_spmd
from concourse.masks import make_identity

B, S, H = 2, 2048, 2048
NH, NKV, HD = 32, 8, 64
SCALE = HD ** -0.5
F32 = mybir.dt.float32
BF16 = mybir.dt.bfloat16

_program_cache = {}
_trace_opts = {}       # test.py may set {"trace": True, "trace_cores": [...], "tmpdir": ...}
_last_results = None   # BassKernelResults of the most recent kernel() call


def _build_program():
    nc = bass.Bass("TRN2", target_bir_lowering=False, debug=False, num_devices=8)

    x_in = nc.dram_tensor("x", [S, H], F32, kind="ExternalInput").ap()
    mask_in = nc.dram_tensor("mask", [S, S], F32, kind="ExternalInput").ap()
    wq_in = nc.dram_tensor("wq", [512, H], F32, kind="ExternalInput").ap()
    wk_in = nc.dram_tensor("wk", [128, H], F32, kind="ExternalInput").ap()
    wv_in = nc.dram_tensor("wv", [128, H], F32, kind="ExternalInput").ap()
    wo_in = nc.dram_tensor("wo", [512, H], F32, kind="ExternalInput").ap()
    out_ext = nc.dram_tensor("out_part", [S, 512], F32, kind="ExternalOutput").ap()

    with tile.TileContext(nc) as tc:
        import contextlib
        with (
            tc.tile_pool(name="persist", bufs=1) as persist,
            tc.tile_pool(name="dram", bufs=1, space="DRAM") as dram,
        ):
          with contextlib.ExitStack() as ctx:
            consts = ctx.enter_context(tc.tile_pool(name="consts", bufs=1))
            wpool = ctx.enter_context(tc.tile_pool(name="wpool", bufs=1))
            qkv = ctx.enter_context(tc.tile_pool(name="qkv", bufs=1))
            big_ps = ctx.enter_context(tc.tile_pool(name="big_ps", bufs=4, space="PSUM"))
            av_pool = ctx.enter_context(tc.tile_pool(name="av_pool", bufs=2, space="PSUM"))
            tr_ps = ctx.enter_context(tc.tile_pool(name="tr_ps", bufs=1, space="PSUM"))

            ident = consts.tile([128, 128], F32)
            make_identity(nc, ident)

            cc_in = dram.tile([4, 128, S], BF16)     # [m, p, s] == rows 128m+p
            cc_out = dram.tile([2048, S], BF16)      # gathered OT (d x s)
            recip_dram = dram.tile([32, 512], F32)

            # ---- persistent sbuf ----
            wqT = wpool.tile([128, 16, 512], BF16)   # [h_in_chunk, h_chunk, qd]
            wkT = wpool.tile([128, 16, 128], BF16)
            wvT = wpool.tile([128, 16, 128], BF16)
            woT = persist.tile([128, 16, 512], BF16)  # [d_in_chunk, d_chunk, i]
            em = wpool.tile([128, 16, 512], BF16)    # expmaskT per (c,d): idx 4c+d
            qt_sb = qkv.tile([128, 4, S], BF16)      # [qd in pair, pair m, s]
            kt_sb = qkv.tile([128, S], BF16)         # [d (2 heads), skv]
            v_sb = qkv.tile([128, 16, 130], BF16)    # [skv in tile, t, V0|1|V1|1]
            ot_sb = qkv.tile([128, 4, S], BF16)      # [d in pair, pair m, s]

            nc.vector.memset(v_sb, 1.0)

            def transpose_in(dst_slice, src_slice, dst_dtype=BF16):
                """PE-transpose a [128,128] f32 sbuf block into dst (copy+cast)."""
                p = tr_ps.tile([128, 128], F32, tag="trps")
                nc.tensor.transpose(p[:], src_slice, ident[:])
                nc.vector.tensor_copy(dst_slice, p[:])

            # ---- weights: load natural f32, transpose via PE ----
            with tc.tile_pool(name="wstage", bufs=2) as wstage:
                for wname, w_in, wT, ncols in (
                    ("wq", wq_in, wqT, 512), ("wo", wo_in, woT, 512),
                    ("wk", wk_in, wkT, 128), ("wv", wv_in, wvT, 128),
                ):
                    nrow = ncols // 128  # 4 or 1 partition-tiles
                    wst = wstage.tile([128, nrow, H], F32, tag="wst", name=f"wst_{wname}")
                    nc.sync.dma_start(out=wst[:], in_=w_in.rearrange("(r p) h -> p r h", p=128))
                    for r in range(nrow):
                        for i in range(16):
                            transpose_in(wT[:, i, r * 128:(r + 1) * 128],
                                         wst[:, r, i * 128:(i + 1) * 128])

                # ---- expmask: diag band of mask, transpose + exp ----
                for c in range(4):
                    mst = wstage.tile([128, 4, 512], F32, tag="mst", name=f"mst_{c}")
                    nc.sync.dma_start(
                        out=mst[:],
                        in_=mask_in[512 * c:512 * (c + 1), 512 * c:512 * (c + 1)]
                            .rearrange("(r p) k -> p r k", p=128))
                    for r in range(4):
                        for d in range(4):
                            p = tr_ps.tile([128, 128], F32, tag="trps")
                            nc.tensor.transpose(p[:], mst[:, r, d * 128:(d + 1) * 128], ident[:])
                            nc.scalar.activation(em[:, 4 * c + d, 128 * r:128 * (r + 1)],
                                                 p[:], mybir.ActivationFunctionType.Exp,
                                                 scale=SCALE)

            # ---- main pipeline over s-chunks ----
            xpool = ctx.enter_context(tc.tile_pool(name="xpool", bufs=1))
            xTpool = ctx.enter_context(tc.tile_pool(name="xTpool", bufs=2))
            probs_pool = ctx.enter_context(tc.tile_pool(name="probs", bufs=4))
            smalls = ctx.enter_context(tc.tile_pool(name="smalls", bufs=4))

            for c in range(4):
                # x chunk natural f32 -> xT bf16
                xnat = xpool.tile([128, 4, H], F32, tag="xnat", name=f"xnat_{c}")
                nc.sync.dma_start(out=xnat[:],
                                  in_=x_in[512 * c:512 * (c + 1), :]
                                  .rearrange("(r p) h -> p r h", p=128))
                xT = xTpool.tile([128, 16, 512], BF16, tag="xT", name=f"xT_{c}")
                for r in range(4):
                    for i in range(16):
                        transpose_in(xT[:, i, r * 128:(r + 1) * 128],
                                     xnat[:, r, i * 128:(i + 1) * 128])

                # QT projection: 4 pair-tiles
                for m in range(4):
                    qp = big_ps.tile([128, 512], F32, tag="bps", name=f"qp_{c}_{m}")
                    for i in range(16):
                        nc.tensor.matmul(qp[:], wqT[:, i, m * 128:(m + 1) * 128],
                                         xT[:, i, :], start=(i == 0), stop=(i == 15))
                    nc.vector.tensor_copy(qt_sb[:, m, 512 * c:512 * (c + 1)], qp[:])
                # KT projection
                kp = big_ps.tile([128, 512], F32, tag="bps", name=f"kp_{c}")
                for i in range(16):
                    nc.tensor.matmul(kp[:], wkT[:, i, :], xT[:, i, :],
                                     start=(i == 0), stop=(i == 15))
                nc.vector.tensor_copy(kt_sb[:, 512 * c:512 * (c + 1)], kp[:])
                # V projection: 4 skv tiles per chunk
                for r in range(4):
                    t = 4 * c + r
                    vp = tr_ps.tile([128, 128], F32, tag="vps", name=f"vp_{t}")
                    for i in range(16):
                        nc.tensor.matmul(vp[:], xT[:, i, r * 128:(r + 1) * 128],
                                         wvT[:, i, :], start=(i == 0), stop=(i == 15))
                    nc.vector.tensor_copy(v_sb[:, t, 0:64], vp[:, 0:64])
                    nc.vector.tensor_copy(v_sb[:, t, 65:129], vp[:, 64:128])

                # ---- attention for this s-chunk ----
                for m in range(4):
                    avA = av_pool.tile([65, 512], F32, tag="av", name=f"avA_{c}_{m}")
                    avB = av_pool.tile([65, 512], F32, tag="av", name=f"avB_{c}_{m}")
                    ntile = 4 * c + 4
                    for t in range(ntile):
                        first, last = (t == 0), (t == ntile - 1)
                        for half, av in ((0, avA), (1, avB)):
                            sp = big_ps.tile([128, 512], F32, tag="bps",
                                             name=f"sp_{c}_{m}_{t}_{half}")
                            nc.tensor.matmul(
                                sp[:], kt_sb[64 * half:64 * half + 64, 128 * t:128 * (t + 1)],
                                qt_sb[64 * half:64 * half + 64, m, 512 * c:512 * (c + 1)],
                                start=True, stop=True, tile_position=(64 * half, 0))
                            pr = probs_pool.tile([128, 512], BF16, tag="pr",
                                                 name=f"pr_{c}_{m}_{t}_{half}")
                            nc.scalar.activation(pr[:], sp[:],
                                                 mybir.ActivationFunctionType.Exp,
                                                 scale=SCALE)
                            if t >= 4 * c:
                                nc.vector.tensor_mul(pr[:], pr[:], em[:, t, :])
                            nc.tensor.matmul(av[:],
                                             v_sb[:, t, 65 * half:65 * half + 65],
                                             pr[:], start=first, stop=last)
                    # divide by rowsum: recip -> dram -> partition-broadcast
                    for half, av in ((0, avA), (1, avB)):
                        u = 8 * c + 2 * m + half
                        rc = smalls.tile([1, 512], F32, tag="rc", name=f"rc_{u}")
                        nc.vector.reciprocal(rc[:], av[64:65, :])
                        nc.sync.dma_start(out=recip_dram[u:u + 1, :], in_=rc[:])
                        bc = smalls.tile([64, 512], F32, tag="bc", name=f"bc_{u}")
                        src = recip_dram[u:u + 1, :].partition_broadcast(64)
                        nc.gpsimd.dma_start(out=bc[:], in_=src)
                        nc.vector.tensor_mul(
                            ot_sb[64 * half:64 * half + 64, m, 512 * c:512 * (c + 1)],
                            av[0:64, :], bc[:])

            # ---- collective: gather OT across the 4-core group ----
            nc.gpsimd.dma_start(out=cc_in.rearrange("m p s -> p m s"), in_=ot_sb[:])
            nc.gpsimd.collective_compute(
                "AllGather", mybir.AluOpType.bypass,
                replica_groups=[[0, 1, 2, 3], [4, 5, 6, 7]],
                ins=[cc_in.opt()], outs=[cc_out.opt()])

          # ---- output projection (phase B pools) ----
          with (
              tc.tile_pool(name="ofull", bufs=1) as ofull,
              tc.tile_pool(name="outst", bufs=3) as outst,
              tc.tile_pool(name="ops", bufs=2, space="PSUM") as ops,
          ):
              of = ofull.tile([128, 16, S], BF16)
              nc.sync.dma_start(out=of[:], in_=cc_out.rearrange("(i p) s -> p i s", p=128))
              for st in range(16):
                  op = ops.tile([128, 512], F32, tag="ops", name=f"op_{st}")
                  for i in range(16):
                      nc.tensor.matmul(op[:], of[:, i, 128 * st:128 * (st + 1)],
                                       woT[:, i, :], start=(i == 0), stop=(i == 15))
                  ost = outst.tile([128, 512], F32, tag="ost", name=f"ost_{st}")
                  nc.vector.tensor_copy(ost[:], op[:])
                  nc.sync.dma_start(out=out_ext[128 * st:128 * (st + 1), :], in_=ost[:])

    _split_excess_waits(nc)
    return nc


def _split_excess_waits(nc, cap=1):
    """Walrus allows few sync-wait slots per instruction; move excess waits
    onto same-engine NoOps placed immediately before (program order keeps
    semantics)."""
    nid = [0]
    for fn in nc.m.functions:
        for bb in fn.blocks:
            insts = list(bb.instructions)
            out = []
            for inst in insts:
                si = getattr(inst, "sync_info", None)
                waits = list(si.on_wait) if si and si.on_wait else []
                if len(waits) > cap:
                    keep = waits[:cap]
                    rest = waits[cap:]
                    while rest:
                        chunk, rest = rest[:cap], rest[cap:]
                        nid[0] += 1
                        nop = mybir.InstNoOp(
                            name=f"waitsplit-{nid[0]}", engine=inst.engine,
                            ins=[], outs=[], bass_nofuse=True,
                            sync_info=mybir.SyncInfo(on_wait=chunk, on_update=[]))
                        out.append(nop)
                    si.on_wait = keep
                out.append(inst)
            bb.instructions[:] = out


def _perm():
    """wo column permutation: gathered-OT row g -> original Wo column."""
    p = np.zeros(2048, np.int64)
    for g in range(2048):
        rank, r = divmod(g, 512)
        m, rr = divmod(r, 128)
        half, d = divmod(rr, 64)
        h = 8 * rank + m + 4 * half
        p[g] = 64 * h + d
    return p


def kernel(hidden_states, attention_mask, Wq, Wk, Wv, Wo):
    hidden_states = np.ascontiguousarray(hidden_states, np.float32)
    attention_mask = np.ascontiguousarray(attention_mask, np.float32)
    Wq = np.asarray(Wq, np.float32); Wk = np.asarray(Wk, np.float32)
    Wv = np.asarray(Wv, np.float32); Wo = np.asarray(Wo, np.float32)

    if "nc" not in _program_cache:
        _program_cache["nc"] = _build_program()
    nc = _program_cache["nc"]

    perm = _perm()
    in_maps = []
    for core in range(8):
        bi, j = divmod(core, 4)
        rows = []
        for m in range(4):
            rows.append(Wq[64 * (8 * j + m):64 * (8 * j + m) + 64])
            rows.append(Wq[64 * (8 * j + 4 + m):64 * (8 * j + 4 + m) + 64])
        in_maps.append({
            "x": np.ascontiguousarray(hidden_states[bi]),
            "mask": np.ascontiguousarray(attention_mask[bi, 0]),
            "wq": np.ascontiguousarray(np.concatenate(rows, 0)),
            "wk": np.ascontiguousarray(Wk[128 * j:128 * (j + 1)]),
            "wv": np.ascontiguousarray(Wv[128 * j:128 * (j + 1)]),
            "wo": np.ascontiguousarray(Wo[512 * j:512 * (j + 1)][:, perm]),
        })

    global _last_results
    res = run_bass_kernel_spmd(nc, in_maps, list(range(8)), **_trace_opts)
    _last_results = res
    out = np.zeros((B, S, H), np.float32)
    for core in range(8):
        bi, j = divmod(core, 4)
        out[bi, :, 512 * j:512 * (j + 1)] = res.results[core]["out_part"]
    return out


if __name__ == "__main__":
    ins = {
        "hidden_states": np.random.randn(B, S, H).astype(np.float32),
        "attention_mask": np.zeros((B, 1, S, S), np.float32),
        "Wq": np.random.randn(2048, H).astype(np.float32) * H ** -0.5,
        "Wk": np.random.randn(512, H).astype(np.float32) * H ** -0.5,
        "Wv": np.random.randn(512, H).astype(np.float32) * H ** -0.5,
        "Wo": np.random.randn(H, 2048).astype(np.float32) * H ** -0.5,
    }
    o = kernel(**ins)
    print("ran", o.shape, o.dtype)



# revision 7
# speedup vs baseline: 1.4722x; 1.4722x over previous
"""GQA kernel for Trainium2, 8 NeuronCores.

Sharding: (batch x kv-head) — cores 0-3 handle batch 0, cores 4-7 batch 1;
each core owns 2 KV heads (8 Q heads). Row-parallel Wo via AllGather of the
attention outputs (bf16) within each 4-core group; each core computes a
512-column slice of the final output.

All layout transposes (x, Wq/Wk/Wv/Wo, mask exp) are done host-side in
numpy, so the device only runs useful matmuls. The Tensor engine is kept
continuously busy (p-state ramp): all QKV projections run first as one
dense block, attention scores are emitted one kv-tile ahead of the AV
accumulation, softmax normalization broadcasts the reciprocal row-sum via a
tiny K=2 PE matmul (no DRAM round-trip), and the AllGather is split into 4
per-s-chunk collectives overlapped with the next chunk's attention.

B=2, S=2048, H=2048, NH=32, NKV=8, HD=64. All matmuls bf16 (f32 PSUM).
Causal structure exploited: fully-masked upper tiles skipped; diagonal
tiles masked multiplicatively with host-computed exp(mask).
"""
import numpy as np
import ml_dtypes

import concourse.bass as bass
import concourse.tile as tile
from concourse import mybir
from concourse.bass_utils import run_bass_kernel_spmd

B, S, H = 2, 2048, 2048
NH, NKV, HD = 32, 8, 64
SCALE = HD ** -0.5
F32 = mybir.dt.float32
BF16 = mybir.dt.bfloat16
BF16_NP = ml_dtypes.bfloat16

_program_cache = {}
_trace_opts = {}       # test.py may set {"trace": True, "trace_cores": [...], "tmpdir": ...}
_last_results = None   # BassKernelResults of the most recent kernel() call


def _build_program():
    nc = bass.Bass("TRN2", target_bir_lowering=False, debug=False, num_devices=8)

    xT_in = nc.dram_tensor("xT", [H, S], BF16, kind="ExternalInput").ap()
    em_in = nc.dram_tensor("em", [128, 16, 512], BF16, kind="ExternalInput").ap()
    wq_in = nc.dram_tensor("wqT", [H, 512], BF16, kind="ExternalInput").ap()
    wk_in = nc.dram_tensor("wkT", [H, 128], BF16, kind="ExternalInput").ap()
    wv_in = nc.dram_tensor("wvT", [H, 128], BF16, kind="ExternalInput").ap()
    wo_in = nc.dram_tensor("woT", [H, 512], BF16, kind="ExternalInput").ap()
    out_ext = nc.dram_tensor("out_part", [S, 512], F32, kind="ExternalOutput").ap()

    with tile.TileContext(nc) as tc:
        import contextlib
        with (
            tc.tile_pool(name="persist", bufs=1) as persist,
            tc.tile_pool(name="dram", bufs=1, space="DRAM") as dram,
        ):
            cc_in = [dram.tile([4, 128, 512], BF16, name=f"cc_in_{c}") for c in range(4)]
            cc_out = [dram.tile([2048, 512], BF16, name=f"cc_out_{c}") for c in range(4)]

            # ---- persistent sbuf ----
            xt = [persist.tile([128, S], BF16, name=f"xt_{i}") for i in range(16)]
            wq_sb = persist.tile([128, 16, 512], BF16)   # [h_in, h_chunk, qd]
            wk_sb = persist.tile([128, 16, 128], BF16)
            wv_sb = persist.tile([128, 16, 128], BF16)
            wo_sb = persist.tile([128, 16, 512], BF16)   # [d_in, d_chunk, hcol]
            em_sb = persist.tile([128, 16, 512], BF16)   # exp(mask)^T per kv tile
            qt_sb = persist.tile([128, 4, S], BF16)      # [qd in pair, pair m, s]
            kt_sb = persist.tile([128, S], BF16)         # [d (2 heads), skv]
            v_sb = persist.tile([128, 16, 130], BF16)    # [skv in tile, t, V0|1|V1|1]
            sel = persist.tile([33, 128], BF16)          # rowsum broadcast selector

            nc.vector.memset(v_sb, 1.0)
            nc.vector.memset(sel, 0.0)
            nc.vector.memset(sel[0:1, 0:64], 1.0)
            nc.vector.memset(sel[32:33, 64:128], 1.0)

            # ---- input DMAs: weights on scalar queue, xT on sync queue ----
            nc.scalar.dma_start(out=wq_sb, in_=wq_in.rearrange("(i p) d -> p i d", p=128))
            for i in range(16):
                nc.sync.dma_start(out=xt[i], in_=xT_in[128 * i:128 * (i + 1), :])
            nc.scalar.dma_start(out=wk_sb, in_=wk_in.rearrange("(i p) d -> p i d", p=128))
            nc.scalar.dma_start(out=wv_sb, in_=wv_in.rearrange("(i p) d -> p i d", p=128))
            nc.scalar.dma_start(out=em_sb, in_=em_in)
            nc.scalar.dma_start(out=wo_sb, in_=wo_in.rearrange("(i p) d -> p i d", p=128))

            # ---- phase P: all projections, dense on PE ----
            with (
                tc.tile_pool(name="proj_ps", bufs=3, space="PSUM") as proj_ps,
                tc.tile_pool(name="v_ps", bufs=2, space="PSUM") as v_ps,
            ):
                for c in range(4):
                    cs = slice(512 * c, 512 * (c + 1))
                    for m in range(4):
                        qp = proj_ps.tile([128, 512], F32, tag="pps", name=f"qp_{c}_{m}")
                        for i in range(16):
                            nc.tensor.matmul(qp[:], wq_sb[:, i, 128 * m:128 * (m + 1)],
                                             xt[i][:, cs], start=(i == 0), stop=(i == 15))
                        nc.vector.tensor_copy(qt_sb[:, m, cs], qp[:])
                    kp = proj_ps.tile([128, 512], F32, tag="pps", name=f"kp_{c}")
                    for i in range(16):
                        nc.tensor.matmul(kp[:], wk_sb[:, i, :], xt[i][:, cs],
                                         start=(i == 0), stop=(i == 15))
                    nc.vector.tensor_copy(kt_sb[:, cs], kp[:])
                    for r in range(4):
                        t = 4 * c + r
                        vp = v_ps.tile([128, 128], F32, tag="vps", name=f"vp_{t}")
                        for i in range(16):
                            nc.tensor.matmul(vp[:], xt[i][:, 128 * t:128 * (t + 1)],
                                             wv_sb[:, i, :], start=(i == 0), stop=(i == 15))
                        nc.vector.tensor_copy(v_sb[:, t, 0:64], vp[:, 0:64])
                        nc.vector.tensor_copy(v_sb[:, t, 65:129], vp[:, 64:128])

            # ---- phase A + O: attention, chunked collective, output proj ----
            with contextlib.ExitStack() as ctx:
                sc_ps = ctx.enter_context(tc.tile_pool(name="sc_ps", bufs=4, space="PSUM"))
                av_ps = ctx.enter_context(tc.tile_pool(name="av_ps", bufs=2, space="PSUM"))
                o_ps = ctx.enter_context(tc.tile_pool(name="o_ps", bufs=2, space="PSUM"))
                probs = ctx.enter_context(tc.tile_pool(name="probs", bufs=6))
                smalls = ctx.enter_context(tc.tile_pool(name="smalls", bufs=4))
                tmp_pool = ctx.enter_context(tc.tile_pool(name="tmp", bufs=2))
                ot_pool = ctx.enter_context(tc.tile_pool(name="ot", bufs=2))
                of_pool = ctx.enter_context(tc.tile_pool(name="of", bufs=2))
                outst = ctx.enter_context(tc.tile_pool(name="outst", bufs=2))

                deferred = []  # normalization tails, emitted after next scores pair

                def flush_deferred():
                    while deferred:
                        deferred.pop(0)()

                def emit_scores(c, m, t):
                    """Scores + exp (+ diag mask) for kv tile t; returns (prA, prB)."""
                    cs = slice(512 * c, 512 * (c + 1))
                    prs = []
                    for half in (0, 1):
                        hp = slice(64 * half, 64 * half + 64)
                        sp = sc_ps.tile([128, 512], F32, tag="bps",
                                        name=f"sp_{c}_{m}_{t}_{half}")
                        nc.tensor.matmul(sp[:], kt_sb[hp, 128 * t:128 * (t + 1)],
                                         qt_sb[hp, m, cs], start=True, stop=True,
                                         tile_position=(64 * half, 0))
                        pr = probs.tile([128, 512], BF16, tag="pr",
                                        name=f"pr_{c}_{m}_{t}_{half}")
                        nc.scalar.activation(pr[:], sp[:],
                                             mybir.ActivationFunctionType.Exp,
                                             scale=SCALE)
                        if t >= 4 * c:
                            nc.vector.tensor_mul(pr[:], pr[:], em_sb[:, t, :])
                        prs.append(pr)
                    # flush pending normalization now that PE has queued work
                    flush_deferred()
                    return prs

                def emit_av(avA, avB, prA, prB, t, first, last):
                    nc.tensor.matmul(avA[:], v_sb[:, t, 0:65], prA[:],
                                     start=first, stop=last)
                    nc.tensor.matmul(avB[:], v_sb[:, t, 65:130], prB[:],
                                     start=first, stop=last)

                def attention_chunk(c, ot_tile):
                    for m in range(4):
                        ntile = 4 * c + 4
                        avA = av_ps.tile([65, 512], F32, tag="av", name=f"avA_{c}_{m}")
                        avB = av_ps.tile([65, 512], F32, tag="av", name=f"avB_{c}_{m}")
                        pend = None
                        for t in range(ntile):
                            prA, prB = emit_scores(c, m, t)
                            if pend is not None:
                                pt, pA, pB = pend
                                emit_av(avA, avB, pA, pB, pt, pt == 0, False)
                            pend = (t, prA, prB)
                        pt, pA, pB = pend
                        emit_av(avA, avB, pA, pB, pt, pt == 0, True)

                        # normalization: recip rowsum, copy av, defer bc-matmul+mul
                        rc = smalls.tile([33, 512], BF16, tag="rc", name=f"rc_{c}_{m}")
                        tmp = tmp_pool.tile([128, 512], F32, tag="tmp",
                                            name=f"tmp_{c}_{m}")
                        nc.vector.memset(rc[:], 1.0)
                        with nc.allow_low_precision(reason="bf16 recip of rowsum"):
                            nc.vector.reciprocal(rc[0:1, :], avA[64:65, :])
                            nc.vector.reciprocal(rc[32:33, :], avB[64:65, :])
                        nc.vector.tensor_copy(tmp[0:64, :], avA[0:64, :])
                        nc.vector.tensor_copy(tmp[64:128, :], avB[0:64, :])

                        def norm_tail(m=m, rc=rc, tmp=tmp):
                            bc = sc_ps.tile([128, 512], F32, tag="bps",
                                            name=f"bc_{c}_{m}")
                            nc.tensor.matmul(bc[:], sel[:], rc[:],
                                             start=True, stop=True)
                            nc.vector.tensor_mul(ot_tile[:, m, :], tmp[:], bc[:])
                        deferred.append(norm_tail)

                def oproj_chunk(c, of_tile):
                    for st in range(4):
                        op = o_ps.tile([128, 512], F32, tag="ops", name=f"op_{c}_{st}")
                        for i in range(16):
                            nc.tensor.matmul(op[:], of_tile[:, i, 128 * st:128 * (st + 1)],
                                             wo_sb[:, i, :], start=(i == 0), stop=(i == 15))
                        ost = outst.tile([128, 512], F32, tag="ost", name=f"ost_{c}_{st}")
                        nc.vector.tensor_copy(ost[:], op[:])
                        nc.scalar.dma_start(out=out_ext[512 * c + 128 * st:
                                                        512 * c + 128 * (st + 1), :],
                                            in_=ost[:])

                of_tiles = {}
                for c in range(4):
                    ot_tile = ot_pool.tile([128, 4, 512], BF16, tag="ot", name=f"ot_{c}")
                    attention_chunk(c, ot_tile)
                    flush_deferred()
                    nc.gpsimd.dma_start(out=cc_in[c].rearrange("m p s -> p m s"),
                                        in_=ot_tile[:])
                    nc.gpsimd.collective_compute(
                        "AllGather", mybir.AluOpType.bypass,
                        replica_groups=[[0, 1, 2, 3], [4, 5, 6, 7]],
                        ins=[cc_in[c].opt()], outs=[cc_out[c].opt()])
                    if c >= 1:
                        cp = c - 1
                        of = of_pool.tile([128, 16, 512], BF16, tag="of",
                                          name=f"of_{cp}")
                        nc.sync.dma_start(out=of[:],
                                          in_=cc_out[cp].rearrange("(i p) s -> p i s", p=128))
                        oproj_chunk(cp, of)
                of = of_pool.tile([128, 16, 512], BF16, tag="of", name="of_3")
                nc.sync.dma_start(out=of[:],
                                  in_=cc_out[3].rearrange("(i p) s -> p i s", p=128))
                oproj_chunk(3, of)

    _split_excess_waits(nc)
    return nc


def _split_excess_waits(nc, cap=1):
    """Walrus allows few sync-wait slots per instruction; move excess waits
    onto same-engine NoOps placed immediately before (program order keeps
    semantics)."""
    nid = [0]
    for fn in nc.m.functions:
        for bb in fn.blocks:
            insts = list(bb.instructions)
            out = []
            for inst in insts:
                si = getattr(inst, "sync_info", None)
                waits = list(si.on_wait) if si and si.on_wait else []
                if len(waits) > cap:
                    keep = waits[:cap]
                    rest = waits[cap:]
                    while rest:
                        chunk, rest = rest[:cap], rest[cap:]
                        nid[0] += 1
                        nop = mybir.InstNoOp(
                            name=f"waitsplit-{nid[0]}", engine=inst.engine,
                            ins=[], outs=[], bass_nofuse=True,
                            sync_info=mybir.SyncInfo(on_wait=chunk, on_update=[]))
                        out.append(nop)
                    si.on_wait = keep
                out.append(inst)
            bb.instructions[:] = out


def _perm():
    """wo column permutation: gathered-OT row g -> original Wo column."""
    p = np.zeros(2048, np.int64)
    for g in range(2048):
        rank, r = divmod(g, 512)
        m, rr = divmod(r, 128)
        half, d = divmod(rr, 64)
        h = 8 * rank + m + 4 * half
        p[g] = 64 * h + d
    return p


def kernel(hidden_states, attention_mask, Wq, Wk, Wv, Wo):
    hidden_states = np.asarray(hidden_states, np.float32)
    attention_mask = np.asarray(attention_mask, np.float32)
    Wq = np.asarray(Wq, np.float32); Wk = np.asarray(Wk, np.float32)
    Wv = np.asarray(Wv, np.float32); Wo = np.asarray(Wo, np.float32)

    if "nc" not in _program_cache:
        _program_cache["nc"] = _build_program()
    nc = _program_cache["nc"]

    perm = _perm()
    # per-batch shared tensors
    xTs, ems = [], []
    for bi in range(B):
        xTs.append(np.ascontiguousarray(hidden_states[bi].T).astype(BF16_NP))
        em = np.zeros((128, 16, 512), np.float32)
        for c in range(4):
            blk = attention_mask[bi, 0, 512 * c:512 * (c + 1), 512 * c:512 * (c + 1)]
            emb = np.exp(blk).T  # [kv, q]
            for dd in range(4):
                em[:, 4 * c + dd, :] = emb[128 * dd:128 * (dd + 1), :]
        ems.append(em.astype(BF16_NP))

    wqTs, wkTs, wvTs, woTs = [], [], [], []
    for j in range(4):
        rows = []
        for m in range(4):
            rows.append(Wq[64 * (8 * j + m):64 * (8 * j + m) + 64])
            rows.append(Wq[64 * (8 * j + 4 + m):64 * (8 * j + 4 + m) + 64])
        wq_core = np.concatenate(rows, 0)
        wqTs.append(np.ascontiguousarray(wq_core.T).astype(BF16_NP))
        wkTs.append(np.ascontiguousarray(Wk[128 * j:128 * (j + 1)].T).astype(BF16_NP))
        wvTs.append(np.ascontiguousarray(Wv[128 * j:128 * (j + 1)].T).astype(BF16_NP))
        wo_core = Wo[512 * j:512 * (j + 1)][:, perm]
        woTs.append(np.ascontiguousarray(wo_core.T).astype(BF16_NP))

    in_maps = []
    for core in range(8):
        bi, j = divmod(core, 4)
        in_maps.append({
            "xT": xTs[bi], "em": ems[bi],
            "wqT": wqTs[j], "wkT": wkTs[j], "wvT": wvTs[j], "woT": woTs[j],
        })

    global _last_results
    res = run_bass_kernel_spmd(nc, in_maps, list(range(8)), **_trace_opts)
    _last_results = res
    out = np.zeros((B, S, H), np.float32)
    for core in range(8):
        bi, j = divmod(core, 4)
        out[bi, :, 512 * j:512 * (j + 1)] = res.results[core]["out_part"]
    return out


if __name__ == "__main__":
    ins = {
        "hidden_states": np.random.randn(B, S, H).astype(np.float32),
        "attention_mask": np.zeros((B, 1, S, S), np.float32),
        "Wq": np.random.randn(2048, H).astype(np.float32) * H ** -0.5,
        "Wk": np.random.randn(512, H).astype(np.float32) * H ** -0.5,
        "Wv": np.random.randn(512, H).astype(np.float32) * H ** -0.5,
        "Wo": np.random.randn(H, 2048).astype(np.float32) * H ** -0.5,
    }
    o = kernel(**ins)
    print("ran", o.shape, o.dtype)


# revision 36
# speedup vs baseline: 1.8504x; 1.2569x over previous
"""GQA kernel for Trainium2, 8 NeuronCores.

Sharding: (batch x kv-head) — cores 0-3 handle batch 0, cores 4-7 batch 1;
each core owns 2 KV heads (8 Q heads). Row-parallel Wo via AllGather of the
attention outputs (bf16) within each 4-core group; each core computes a
512-column slice of the final output.

All layout transposes (x, Wq/Wk/Wv/Wo, mask exp) are done host-side in
numpy, so the device only runs useful matmuls. The Tensor engine is kept
continuously busy (p-state ramp): all QKV projections run first as one
dense block, attention scores are emitted one kv-tile ahead of the AV
accumulation, softmax normalization broadcasts the reciprocal row-sum via a
tiny K=2 PE matmul (no DRAM round-trip), and the AllGather is split into 4
per-s-chunk collectives overlapped with the next chunk's attention.

B=2, S=2048, H=2048, NH=32, NKV=8, HD=64. All matmuls bf16 (f32 PSUM).
Causal structure exploited: fully-masked upper tiles skipped; diagonal
tiles masked multiplicatively with host-computed exp(mask).
"""
import numpy as np
import ml_dtypes

import concourse.bass as bass
import concourse.tile as tile
from concourse import library_config, mybir
from concourse.bass_utils import run_bass_kernel_spmd

B, S, H = 2, 2048, 2048
NH, NKV, HD = 32, 8, 64
SCALE = HD ** -0.5
F32 = mybir.dt.float32
BF16 = mybir.dt.bfloat16
BF16_NP = ml_dtypes.bfloat16

_program_cache = {}
_trace_opts = {}       # test.py may set {"trace": True, "trace_cores": [...], "tmpdir": ...}
_last_results = None   # BassKernelResults of the most recent kernel() call


def _build_program():
    nc = bass.Bass("TRN2", target_bir_lowering=False, debug=False, num_devices=8)

    xT_in = nc.dram_tensor("xT", [H, S], BF16, kind="ExternalInput").ap()
    em_in = nc.dram_tensor("em", [128, 16, 512], BF16, kind="ExternalInput").ap()
    wq_in = nc.dram_tensor("wqT", [H, 512], BF16, kind="ExternalInput").ap()
    wk_in = nc.dram_tensor("wkT", [H, 128], BF16, kind="ExternalInput").ap()
    wv_in = nc.dram_tensor("wvT", [H, 128], BF16, kind="ExternalInput").ap()
    wo_in = nc.dram_tensor("woT", [H, 512], BF16, kind="ExternalInput").ap()
    out_ext = nc.dram_tensor("out_part", [S, 512], F32, kind="ExternalOutput").ap()

    with tile.TileContext(nc) as tc:
        import contextlib
        with (
            tc.tile_pool(name="persist", bufs=1) as persist,
            tc.tile_pool(name="dram", bufs=1, space="DRAM") as dram,
        ):
            cc_in = [dram.tile([4, 128, 512], BF16, name=f"cc_in_{c}") for c in range(4)]
            cc_out = [dram.tile([2048, 512], BF16, name=f"cc_out_{c}") for c in range(4)]
            rc_dram = dram.tile([32, 512], F32)   # recip bounce: [2*(4c+m)+half, q]

            # ---- persistent sbuf ----
            xt = [persist.tile([128, S], BF16, name=f"xt_{i}") for i in range(16)]
            wq_sb = persist.tile([128, 16, 512], BF16)   # [h_in, h_chunk, qd]
            wk_sb = persist.tile([128, 16, 128], BF16)
            wv_sb = persist.tile([128, 16, 128], BF16)
            wo_sb = persist.tile([128, 16, 512], BF16)   # [d_in, d_chunk, hcol]
            em_sb = persist.tile([128, 16, 512], BF16)   # exp(mask)^T per kv tile
            qt_sb = persist.tile([128, 4, S], BF16)      # [qd in pair, pair m, s]
            kt_sb = persist.tile([128, S], BF16)         # [d (2 heads), skv]
            v_sb = persist.tile([128, 16, 130], BF16)    # [skv in tile, t, V0|1|V1|1]
            nc.vector.memset(v_sb, 1.0)

            # ---- input DMAs: weights on scalar queue, xT on sync queue ----
            nc.scalar.dma_start(out=wq_sb, in_=wq_in.rearrange("(i p) d -> p i d", p=128))
            for i in range(16):
                nc.sync.dma_start(out=xt[i], in_=xT_in[128 * i:128 * (i + 1), :])
            nc.scalar.dma_start(out=wk_sb, in_=wk_in.rearrange("(i p) d -> p i d", p=128))
            nc.scalar.dma_start(out=wv_sb, in_=wv_in.rearrange("(i p) d -> p i d", p=128))
            nc.scalar.dma_start(out=em_sb, in_=em_in)
            nc.scalar.dma_start(out=wo_sb, in_=wo_in.rearrange("(i p) d -> p i d", p=128))

            # ---- phase P: all projections, dense on PE ----
            with (
                tc.tile_pool(name="proj_ps", bufs=3, space="PSUM") as proj_ps,
                tc.tile_pool(name="v_ps", bufs=2, space="PSUM") as v_ps,
            ):
                for c in range(4):
                    cs = slice(512 * c, 512 * (c + 1))
                    for m in range(4):
                        qp = proj_ps.tile([128, 512], F32, tag="pps", name=f"qp_{c}_{m}")
                        for i in range(16):
                            nc.tensor.matmul(qp[:], wq_sb[:, i, 128 * m:128 * (m + 1)],
                                             xt[i][:, cs], start=(i == 0), stop=(i == 15))
                        nc.vector.tensor_copy(qt_sb[:, m, cs], qp[:])
                    kp = proj_ps.tile([128, 512], F32, tag="pps", name=f"kp_{c}")
                    for i in range(16):
                        nc.tensor.matmul(kp[:], wk_sb[:, i, :], xt[i][:, cs],
                                         start=(i == 0), stop=(i == 15))
                    nc.vector.tensor_copy(kt_sb[:, cs], kp[:])
                    for r in range(4):
                        t = 4 * c + r
                        vp = v_ps.tile([128, 128], F32, tag="vps", name=f"vp_{t}")
                        for i in range(16):
                            nc.tensor.matmul(vp[:], xt[i][:, 128 * t:128 * (t + 1)],
                                             wv_sb[:, i, :], start=(i == 0), stop=(i == 15))
                        nc.vector.tensor_copy(v_sb[:, t, 0:64], vp[:, 0:64])
                        nc.vector.tensor_copy(v_sb[:, t, 65:129], vp[:, 64:128])

            # ---- phase A + O: attention, chunked collective, output proj ----
            with contextlib.ExitStack() as ctx:
                sc_ps = ctx.enter_context(tc.tile_pool(name="sc_ps", bufs=2, space="PSUM"))
                av_ps = ctx.enter_context(tc.tile_pool(name="av_ps", bufs=2, space="PSUM"))
                o_ps = ctx.enter_context(tc.tile_pool(name="o_ps", bufs=2, space="PSUM"))
                bc_pool = ctx.enter_context(tc.tile_pool(name="bc", bufs=2))
                probs = ctx.enter_context(tc.tile_pool(name="probs", bufs=4))
                smalls = ctx.enter_context(tc.tile_pool(name="smalls", bufs=2))
                tmp_pool = ctx.enter_context(tc.tile_pool(name="tmp", bufs=2))
                ot_pool = ctx.enter_context(tc.tile_pool(name="ot", bufs=2))
                of_pool = ctx.enter_context(tc.tile_pool(name="of", bufs=1))
                outst = ctx.enter_context(tc.tile_pool(name="outst", bufs=2))

                deferred = []  # (due_seq, fn) normalization tails
                seq = [0]

                def flush_deferred(all_=False):
                    while deferred and (all_ or deferred[0][0] <= seq[0]):
                        deferred.pop(0)[1]()

                def emit_scores(c, m, t):
                    """Scores + fused-pair exp (+ diag mask) for kv tile t."""
                    cs = slice(512 * c, 512 * (c + 1))
                    sp = sc_ps.tile([128, 1024], F32, tag="sp2",
                                    name=f"sp_{c}_{m}_{t}")
                    for half in (0, 1):
                        hp = slice(64 * half, 64 * half + 64)
                        nc.tensor.matmul(sp[:, 512 * half:512 * (half + 1)],
                                         kt_sb[hp, 128 * t:128 * (t + 1)],
                                         qt_sb[hp, m, cs], start=True, stop=True,
                                         tile_position=(64 * half, 0))
                    pr = probs.tile([128, 1024], BF16, tag="pr",
                                    name=f"pr_{c}_{m}_{t}")
                    nc.scalar.activation(pr[:], sp[:],
                                         mybir.ActivationFunctionType.Exp,
                                         scale=SCALE)
                    if t >= 4 * c:
                        nc.vector.tensor_mul(pr[:, 0:512], pr[:, 0:512],
                                             em_sb[:, t, :])
                        nc.vector.tensor_mul(pr[:, 512:1024], pr[:, 512:1024],
                                             em_sb[:, t, :])
                    # flush any normalization tails that are due
                    seq[0] += 1
                    flush_deferred()
                    return pr

                def emit_av(avA, avB, pr, t, first, last):
                    nc.tensor.matmul(avA[:], v_sb[:, t, 0:65], pr[:, 0:512],
                                     start=first, stop=last)
                    nc.tensor.matmul(avB[:], v_sb[:, t, 65:130], pr[:, 512:1024],
                                     start=first, stop=last)

                def attention_chunk(c, ot_tile):
                    for m in range(4):
                        ntile = 4 * c + 4
                        avA = av_ps.tile([65, 512], F32, tag="av", name=f"avA_{c}_{m}")
                        avB = av_ps.tile([65, 512], F32, tag="av", name=f"avB_{c}_{m}")
                        pend = None
                        for t in range(ntile):
                            pr = emit_scores(c, m, t)
                            if pend is not None:
                                pt, ppr = pend
                                emit_av(avA, avB, ppr, pt, pt == 0, False)
                            pend = (t, pr)
                        pt, ppr = pend
                        emit_av(avA, avB, ppr, pt, pt == 0, True)

                        # normalization, all off the PE/scalar critical path:
                        # copy av + sum rows out (vector), DMA-bounce broadcast
                        # the sums, then divide on gpsimd
                        sums = smalls.tile([33, 512], F32, tag="rc",
                                           name=f"sums_{c}_{m}")
                        tmp = tmp_pool.tile([128, 512], F32, tag="tmp",
                                            name=f"tmp_{c}_{m}")
                        bc = bc_pool.tile([128, 512], F32, tag="bc",
                                          name=f"bc_{c}_{m}")
                        rcp = smalls.tile([33, 512], F32, tag="rcp",
                                          name=f"rcp_{c}_{m}")
                        nc.vector.tensor_copy(tmp[0:64, :], avA[0:64, :])
                        nc.vector.tensor_copy(tmp[64:128, :], avB[0:64, :])
                        nc.vector.tensor_copy(sums[0:1, :], avA[64:65, :])
                        nc.vector.tensor_copy(sums[32:33, :], avB[64:65, :])
                        nc.vector.reciprocal(rcp[:], sums[:])
                        u = 2 * (4 * c + m)
                        nc.gpsimd.dma_start(out=rc_dram[u:u + 1, :], in_=rcp[0:1, :])
                        nc.gpsimd.dma_start(out=rc_dram[u + 1:u + 2, :],
                                            in_=rcp[32:33, :])
                        nc.gpsimd.dma_start(
                            out=bc[0:64, :],
                            in_=rc_dram[u:u + 1, :].partition_broadcast(64))
                        nc.gpsimd.dma_start(
                            out=bc[64:128, :],
                            in_=rc_dram[u + 1:u + 2, :].partition_broadcast(64))
                        nc.gpsimd.tensor_mul(ot_tile[:, m, :], tmp[:], bc[:])

                def oproj_chunk(c, of_tile):
                    for st in range(4):
                        op = o_ps.tile([128, 512], F32, tag="ops", name=f"op_{c}_{st}")
                        for i in range(16):
                            nc.tensor.matmul(op[:], of_tile[:, i, 128 * st:128 * (st + 1)],
                                             wo_sb[:, i, :], start=(i == 0), stop=(i == 15))
                        ost = outst.tile([128, 512], F32, tag="ost", name=f"ost_{c}_{st}")
                        nc.vector.tensor_copy(ost[:], op[:])
                        nc.scalar.dma_start(out=out_ext[512 * c + 128 * st:
                                                        512 * c + 128 * (st + 1), :],
                                            in_=ost[:])

                of_tiles = {}
                for c in range(4):
                    ot_tile = ot_pool.tile([128, 4, 512], BF16, tag="ot", name=f"ot_{c}")
                    attention_chunk(c, ot_tile)
                    flush_deferred(all_=True)
                    nc.gpsimd.dma_start(out=cc_in[c].rearrange("m p s -> p m s"),
                                        in_=ot_tile[:])
                    nc.gpsimd.collective_compute(
                        "AllGather", mybir.AluOpType.bypass,
                        replica_groups=[[0, 1, 2, 3], [4, 5, 6, 7]],
                        ins=[cc_in[c].opt()], outs=[cc_out[c].opt()])
                    if c >= 1:
                        cp = c - 1
                        of = of_pool.tile([128, 16, 512], BF16, tag="of",
                                          name=f"of_{cp}")
                        nc.sync.dma_start(out=of[:],
                                          in_=cc_out[cp].rearrange("(i p) s -> p i s", p=128))
                        oproj_chunk(cp, of)
                of = of_pool.tile([128, 16, 512], BF16, tag="of", name="of_3")
                nc.sync.dma_start(out=of[:],
                                  in_=cc_out[3].rearrange("(i p) s -> p i s", p=128))
                oproj_chunk(3, of)

    _split_excess_waits(nc)
    return nc


def _split_excess_waits(nc, cap=1):
    """Walrus allows few sync-wait slots per instruction; move excess waits
    onto same-engine NoOps placed immediately before (program order keeps
    semantics)."""
    nid = [0]
    for fn in nc.m.functions:
        for bb in fn.blocks:
            insts = list(bb.instructions)
            out = []
            for inst in insts:
                si = getattr(inst, "sync_info", None)
                waits = list(si.on_wait) if si and si.on_wait else []
                if len(waits) > cap:
                    keep = waits[:cap]
                    rest = waits[cap:]
                    while rest:
                        chunk, rest = rest[:cap], rest[cap:]
                        nid[0] += 1
                        nop = mybir.InstNoOp(
                            name=f"waitsplit-{nid[0]}", engine=inst.engine,
                            ins=[], outs=[], bass_nofuse=True,
                            sync_info=mybir.SyncInfo(on_wait=chunk, on_update=[]))
                        out.append(nop)
                    si.on_wait = keep
                out.append(inst)
            bb.instructions[:] = out


def _perm():
    """wo column permutation: gathered-OT row g -> original Wo column."""
    p = np.zeros(2048, np.int64)
    for g in range(2048):
        rank, r = divmod(g, 512)
        m, rr = divmod(r, 128)
        half, d = divmod(rr, 64)
        h = 8 * rank + m + 4 * half
        p[g] = 64 * h + d
    return p


def kernel(hidden_states, attention_mask, Wq, Wk, Wv, Wo):
    hidden_states = np.asarray(hidden_states, np.float32)
    attention_mask = np.asarray(attention_mask, np.float32)
    Wq = np.asarray(Wq, np.float32); Wk = np.asarray(Wk, np.float32)
    Wv = np.asarray(Wv, np.float32); Wo = np.asarray(Wo, np.float32)

    if "nc" not in _program_cache:
        _program_cache["nc"] = _build_program()
    nc = _program_cache["nc"]

    perm = _perm()
    # per-batch shared tensors
    xTs, ems = [], []
    for bi in range(B):
        xTs.append(np.ascontiguousarray(hidden_states[bi].T).astype(BF16_NP))
        em = np.zeros((128, 16, 512), np.float32)
        for c in range(4):
            blk = attention_mask[bi, 0, 512 * c:512 * (c + 1), 512 * c:512 * (c + 1)]
            emb = np.exp(blk).T  # [kv, q]
            for dd in range(4):
                em[:, 4 * c + dd, :] = emb[128 * dd:128 * (dd + 1), :]
        ems.append(em.astype(BF16_NP))

    wqTs, wkTs, wvTs, woTs = [], [], [], []
    for j in range(4):
        rows = []
        for m in range(4):
            rows.append(Wq[64 * (8 * j + m):64 * (8 * j + m) + 64])
            rows.append(Wq[64 * (8 * j + 4 + m):64 * (8 * j + 4 + m) + 64])
        wq_core = np.concatenate(rows, 0)
        wqTs.append(np.ascontiguousarray(wq_core.T).astype(BF16_NP))
        wkTs.append(np.ascontiguousarray(Wk[128 * j:128 * (j + 1)].T).astype(BF16_NP))
        wvTs.append(np.ascontiguousarray(Wv[128 * j:128 * (j + 1)].T).astype(BF16_NP))
        wo_core = Wo[512 * j:512 * (j + 1)][:, perm]
        woTs.append(np.ascontiguousarray(wo_core.T).astype(BF16_NP))

    in_maps = []
    for core in range(8):
        bi, j = divmod(core, 4)
        in_maps.append({
            "xT": xTs[bi], "em": ems[bi],
            "wqT": wqTs[j], "wkT": wkTs[j], "wvT": wvTs[j], "woT": woTs[j],
        })

    global _last_results
    res = run_bass_kernel_spmd(nc, in_maps, list(range(8)), **_trace_opts)
    _last_results = res
    out = np.zeros((B, S, H), np.float32)
    for core in range(8):
        bi, j = divmod(core, 4)
        out[bi, :, 512 * j:512 * (j + 1)] = res.results[core]["out_part"]
    return out


if __name__ == "__main__":
    ins = {
        "hidden_states": np.random.randn(B, S, H).astype(np.float32),
        "attention_mask": np.zeros((B, 1, S, S), np.float32),
        "Wq": np.random.randn(2048, H).astype(np.float32) * H ** -0.5,
        "Wk": np.random.randn(512, H).astype(np.float32) * H ** -0.5,
        "Wv": np.random.randn(512, H).astype(np.float32) * H ** -0.5,
        "Wo": np.random.randn(H, 2048).astype(np.float32) * H ** -0.5,
    }
    o = kernel(**ins)
    print("ran", o.shape, o.dtype)


# revision 45
# speedup vs baseline: 2.0597x; 1.1131x over previous
"""GQA kernel for Trainium2, 8 NeuronCores.

Sharding: (batch x kv-head) — cores 0-3 handle batch 0, cores 4-7 batch 1;
each core owns 2 KV heads (8 Q heads). Row-parallel Wo via AllGather of the
attention outputs (bf16) within each 4-core group; each core computes a
512-column slice of the final output.

All layout transposes (x, Wq/Wk/Wv/Wo, mask exp) are done host-side in
numpy, so the device only runs useful matmuls. The Tensor engine is kept
continuously busy (p-state ramp): all QKV projections run first as one
dense block, attention scores are emitted one kv-tile ahead of the AV
accumulation, softmax normalization broadcasts the reciprocal row-sum via a
tiny K=2 PE matmul (no DRAM round-trip), and the AllGather is split into 4
per-s-chunk collectives overlapped with the next chunk's attention.

B=2, S=2048, H=2048, NH=32, NKV=8, HD=64. All matmuls bf16 (f32 PSUM).
Causal structure exploited: fully-masked upper tiles skipped; diagonal
tiles masked multiplicatively with host-computed exp(mask).
"""
import numpy as np
import ml_dtypes

import concourse.bass as bass
import concourse.tile as tile
from concourse import library_config, mybir
from concourse.bass_utils import run_bass_kernel_spmd

B, S, H = 2, 2048, 2048
NH, NKV, HD = 32, 8, 64
SCALE = HD ** -0.5
F32 = mybir.dt.float32
BF16 = mybir.dt.bfloat16
BF16_NP = ml_dtypes.bfloat16

_program_cache = {}
_trace_opts = {}       # test.py may set {"trace": True, "trace_cores": [...], "tmpdir": ...}
_last_results = None   # BassKernelResults of the most recent kernel() call


def _build_program():
    nc = bass.Bass("TRN2", target_bir_lowering=False, debug=False, num_devices=8)

    xT_in = nc.dram_tensor("xT", [H, S], BF16, kind="ExternalInput").ap()
    em_in = nc.dram_tensor("em", [128, 16, 512], BF16, kind="ExternalInput").ap()
    wq_in = nc.dram_tensor("wqT", [H, 512], BF16, kind="ExternalInput").ap()
    wk_in = nc.dram_tensor("wkT", [H, 128], BF16, kind="ExternalInput").ap()
    wv_in = nc.dram_tensor("wvT", [H, 128], BF16, kind="ExternalInput").ap()
    wo_in = nc.dram_tensor("woT", [H, 512], BF16, kind="ExternalInput").ap()
    out_ext = nc.dram_tensor("out_part", [S, 512], F32, kind="ExternalOutput").ap()

    with tile.TileContext(nc) as tc:
        import contextlib
        with (
            tc.tile_pool(name="persist", bufs=1) as persist,
            tc.tile_pool(name="dram", bufs=1, space="DRAM") as dram,
        ):
            cc_in = [dram.tile([128, 512], BF16, name=f"cc_in_{u}") for u in range(16)]
            cc_out = [dram.tile([512, 512], BF16, name=f"cc_out_{u}") for u in range(16)]
            warm_in = dram.tile([4, 64], BF16)
            warm_out = dram.tile([16, 64], BF16)
            rc_dram = dram.tile([32, 512], F32)   # recip bounce: [2*(4c+m)+half, q]

            # ---- persistent sbuf ----
            wq_sb = persist.tile([128, 16, 512], BF16)   # [h_in, h_chunk, qd]
            wk_sb = persist.tile([128, 16, 128], BF16)
            wv_sb = persist.tile([128, 16, 128], BF16)
            wo_sb = persist.tile([128, 16, 512], BF16)   # [d_in, d_chunk, hcol]
            em_sb = persist.tile([128, 16, 512], BF16)   # exp(mask)^T per kv tile
            qt_sb = persist.tile([128, 4, S], BF16)      # [qd in pair, pair m, s]
            kt_sb = persist.tile([128, S], BF16)         # [d (2 heads), skv]
            v_sb = persist.tile([128, 16, 130], BF16)    # [skv in tile, t, V0|1|V1|1]
            nc.vector.memset(v_sb, 1.0)

            # warm up the collective rings with a tiny AllGather
            warm_sb = persist.tile([4, 64], BF16)
            nc.vector.memset(warm_sb, 0.0)
            nc.gpsimd.dma_start(out=warm_in, in_=warm_sb[:])
            nc.gpsimd.collective_compute(
                "AllGather", mybir.AluOpType.bypass,
                replica_groups=[[0, 1, 2, 3], [4, 5, 6, 7]],
                ins=[warm_in.opt()], outs=[warm_out.opt()])

            # ---- phase P: all projections, dense on PE ----
            # xt lives only in this scope; its 64KB/partition is reused by
            # the phase A/O pools afterwards.
            with (
                tc.tile_pool(name="xtpool", bufs=1) as xtpool,
                tc.tile_pool(name="proj_ps", bufs=3, space="PSUM") as proj_ps,
                tc.tile_pool(name="v_ps", bufs=2, space="PSUM") as v_ps,
            ):
                xt = [xtpool.tile([128, S], BF16, name=f"xt_{i}") for i in range(16)]
                # input DMAs split across both HWDGE queues
                nc.scalar.dma_start(out=wq_sb,
                                    in_=wq_in.rearrange("(i p) d -> p i d", p=128))
                for i in range(16):
                    eng = nc.sync if i % 2 == 0 else nc.scalar
                    eng.dma_start(out=xt[i], in_=xT_in[128 * i:128 * (i + 1), :])
                nc.sync.dma_start(out=wk_sb,
                                  in_=wk_in.rearrange("(i p) d -> p i d", p=128))
                nc.sync.dma_start(out=wv_sb,
                                  in_=wv_in.rearrange("(i p) d -> p i d", p=128))
                nc.scalar.dma_start(out=em_sb, in_=em_in)
                nc.sync.dma_start(out=wo_sb,
                                  in_=wo_in.rearrange("(i p) d -> p i d", p=128))
                for c in range(4):
                    cs = slice(512 * c, 512 * (c + 1))
                    for m in range(4):
                        qp = proj_ps.tile([128, 512], F32, tag="pps", name=f"qp_{c}_{m}")
                        for i in range(16):
                            nc.tensor.matmul(qp[:], wq_sb[:, i, 128 * m:128 * (m + 1)],
                                             xt[i][:, cs], start=(i == 0), stop=(i == 15))
                        nc.vector.tensor_copy(qt_sb[:, m, cs], qp[:])
                    kp = proj_ps.tile([128, 512], F32, tag="pps", name=f"kp_{c}")
                    for i in range(16):
                        nc.tensor.matmul(kp[:], wk_sb[:, i, :], xt[i][:, cs],
                                         start=(i == 0), stop=(i == 15))
                    nc.vector.tensor_copy(kt_sb[:, cs], kp[:])
                    for r in range(4):
                        t = 4 * c + r
                        vp = v_ps.tile([128, 128], F32, tag="vps", name=f"vp_{t}")
                        for i in range(16):
                            nc.tensor.matmul(vp[:], xt[i][:, 128 * t:128 * (t + 1)],
                                             wv_sb[:, i, :], start=(i == 0), stop=(i == 15))
                        nc.vector.tensor_copy(v_sb[:, t, 0:64], vp[:, 0:64])
                        nc.vector.tensor_copy(v_sb[:, t, 65:129], vp[:, 64:128])

            # ---- phase A + O: attention, chunked collective, output proj ----
            with contextlib.ExitStack() as ctx:
                sc_ps = ctx.enter_context(tc.tile_pool(name="sc_ps", bufs=2, space="PSUM"))
                av_ps = ctx.enter_context(tc.tile_pool(name="av_ps", bufs=2, space="PSUM"))
                o_ps = ctx.enter_context(tc.tile_pool(name="o_ps", bufs=2, space="PSUM"))
                bc_pool = ctx.enter_context(tc.tile_pool(name="bc", bufs=2))
                probs = ctx.enter_context(tc.tile_pool(name="probs", bufs=4))
                smalls = ctx.enter_context(tc.tile_pool(name="smalls", bufs=2))
                tmp_pool = ctx.enter_context(tc.tile_pool(name="tmp", bufs=2))
                ot_pool = ctx.enter_context(tc.tile_pool(name="ot", bufs=2))
                of_pool = ctx.enter_context(tc.tile_pool(name="of", bufs=2))
                outst = ctx.enter_context(tc.tile_pool(name="outst", bufs=2))

                deferred = []  # (due_seq, fn) normalization tails
                seq = [0]

                def flush_deferred(all_=False):
                    while deferred and (all_ or deferred[0][0] <= seq[0]):
                        deferred.pop(0)[1]()

                def emit_scores(c, m, t):
                    """Scores + fused-pair exp (+ diag mask) for kv tile t."""
                    cs = slice(512 * c, 512 * (c + 1))
                    sp = sc_ps.tile([128, 1024], F32, tag="sp2",
                                    name=f"sp_{c}_{m}_{t}")
                    for half in (0, 1):
                        hp = slice(64 * half, 64 * half + 64)
                        nc.tensor.matmul(sp[:, 512 * half:512 * (half + 1)],
                                         kt_sb[hp, 128 * t:128 * (t + 1)],
                                         qt_sb[hp, m, cs], start=True, stop=True,
                                         tile_position=(64 * half, 0))
                    pr = probs.tile([128, 1024], BF16, tag="pr",
                                    name=f"pr_{c}_{m}_{t}")
                    nc.scalar.activation(pr[:], sp[:],
                                         mybir.ActivationFunctionType.Exp,
                                         scale=SCALE)
                    if t >= 4 * c:
                        nc.vector.tensor_mul(pr[:, 0:512], pr[:, 0:512],
                                             em_sb[:, t, :])
                        nc.vector.tensor_mul(pr[:, 512:1024], pr[:, 512:1024],
                                             em_sb[:, t, :])
                    # flush any normalization tails that are due
                    seq[0] += 1
                    flush_deferred()
                    return pr

                def emit_av(avA, avB, pr, t, first, last):
                    nc.tensor.matmul(avA[:], v_sb[:, t, 0:65], pr[:, 0:512],
                                     start=first, stop=last)
                    nc.tensor.matmul(avB[:], v_sb[:, t, 65:130], pr[:, 512:1024],
                                     start=first, stop=last)

                def attention_chunk(c, ot_tile, of_tile):
                    for m in range(4):
                        ntile = 4 * c + 4
                        avA = av_ps.tile([65, 512], F32, tag="av", name=f"avA_{c}_{m}")
                        avB = av_ps.tile([65, 512], F32, tag="av", name=f"avB_{c}_{m}")
                        pend = None
                        for t in range(ntile):
                            pr = emit_scores(c, m, t)
                            if pend is not None:
                                pt, ppr = pend
                                emit_av(avA, avB, ppr, pt, pt == 0, False)
                            pend = (t, pr)
                        pt, ppr = pend
                        emit_av(avA, avB, ppr, pt, pt == 0, True)

                        # normalization, all off the PE/scalar critical path:
                        # copy av + sum rows out (vector), DMA-bounce broadcast
                        # the sums, then divide on gpsimd
                        sums = smalls.tile([33, 512], F32, tag="rc",
                                           name=f"sums_{c}_{m}")
                        tmp = tmp_pool.tile([128, 512], F32, tag="tmp",
                                            name=f"tmp_{c}_{m}")
                        bc = bc_pool.tile([128, 512], F32, tag="bc",
                                          name=f"bc_{c}_{m}")
                        rcp = smalls.tile([33, 512], F32, tag="rcp",
                                          name=f"rcp_{c}_{m}")
                        nc.vector.tensor_copy(sums[0:1, :], avA[64:65, :])
                        nc.vector.tensor_copy(sums[32:33, :], avB[64:65, :])
                        nc.vector.reciprocal(rcp[:], sums[:])
                        nc.vector.tensor_copy(tmp[0:64, :], avA[0:64, :])
                        nc.vector.tensor_copy(tmp[64:128, :], avB[0:64, :])
                        u = 2 * (4 * c + m)
                        nc.gpsimd.dma_start(out=rc_dram[u:u + 1, :], in_=rcp[0:1, :])
                        nc.gpsimd.dma_start(out=rc_dram[u + 1:u + 2, :],
                                            in_=rcp[32:33, :])
                        nc.gpsimd.dma_start(
                            out=bc[0:64, :],
                            in_=rc_dram[u:u + 1, :].partition_broadcast(64))
                        nc.gpsimd.dma_start(
                            out=bc[64:128, :],
                            in_=rc_dram[u + 1:u + 2, :].partition_broadcast(64))
                        nc.gpsimd.tensor_mul(ot_tile[:, m, :], tmp[:], bc[:])

                        # per-(c,m) collective: gather this head-pair's OT
                        # across the 4-core group, load into of as it lands
                        uu = 4 * c + m
                        nc.gpsimd.dma_start(out=cc_in[uu], in_=ot_tile[:, m, :])
                        nc.gpsimd.collective_compute(
                            "AllGather", mybir.AluOpType.bypass,
                            replica_groups=[[0, 1, 2, 3], [4, 5, 6, 7]],
                            ins=[cc_in[uu].opt()], outs=[cc_out[uu].opt()])
                        nc.sync.dma_start(
                            out=of_tile[:, 4 * m:4 * (m + 1), :],
                            in_=cc_out[uu].rearrange("(r p) s -> p r s", p=128))

                def oproj_chunk(c, of_tile):
                    for st in range(4):
                        op = o_ps.tile([128, 512], F32, tag="ops", name=f"op_{c}_{st}")
                        for i in range(16):
                            nc.tensor.matmul(op[:], of_tile[:, i, 128 * st:128 * (st + 1)],
                                             wo_sb[:, i, :], start=(i == 0), stop=(i == 15))
                        ost = outst.tile([128, 512], F32, tag="ost", name=f"ost_{c}_{st}")
                        nc.vector.tensor_copy(ost[:], op[:])
                        nc.scalar.dma_start(out=out_ext[512 * c + 128 * st:
                                                        512 * c + 128 * (st + 1), :],
                                            in_=ost[:])

                ofs = {}
                for c in range(4):
                    ot_tile = ot_pool.tile([128, 4, 512], BF16, tag="ot", name=f"ot_{c}")
                    ofs[c] = of_pool.tile([128, 16, 512], BF16, tag="of",
                                          name=f"of_{c}")
                    attention_chunk(c, ot_tile, ofs[c])
                    flush_deferred(all_=True)
                    if c >= 1:
                        oproj_chunk(c - 1, ofs[c - 1])
                oproj_chunk(3, ofs[3])

    _split_excess_waits(nc)
    return nc


def _split_excess_waits(nc, cap=1):
    """Walrus allows few sync-wait slots per instruction; move excess waits
    onto same-engine NoOps placed immediately before (program order keeps
    semantics)."""
    nid = [0]
    for fn in nc.m.functions:
        for bb in fn.blocks:
            insts = list(bb.instructions)
            out = []
            for inst in insts:
                si = getattr(inst, "sync_info", None)
                waits = list(si.on_wait) if si and si.on_wait else []
                if len(waits) > cap:
                    keep = waits[:cap]
                    rest = waits[cap:]
                    while rest:
                        chunk, rest = rest[:cap], rest[cap:]
                        nid[0] += 1
                        nop = mybir.InstNoOp(
                            name=f"waitsplit-{nid[0]}", engine=inst.engine,
                            ins=[], outs=[], bass_nofuse=True,
                            sync_info=mybir.SyncInfo(on_wait=chunk, on_update=[]))
                        out.append(nop)
                    si.on_wait = keep
                out.append(inst)
            bb.instructions[:] = out


def _perm():
    """wo column permutation: gathered-OT row g -> original Wo column.

    Per-(c,m) AllGather layout: g = 512*m + 128*rank + 64*half + d."""
    p = np.zeros(2048, np.int64)
    for g in range(2048):
        m, r = divmod(g, 512)
        rank, rr = divmod(r, 128)
        half, d = divmod(rr, 64)
        h = 8 * rank + m + 4 * half
        p[g] = 64 * h + d
    return p


def kernel(hidden_states, attention_mask, Wq, Wk, Wv, Wo):
    hidden_states = np.asarray(hidden_states, np.float32)
    attention_mask = np.asarray(attention_mask, np.float32)
    Wq = np.asarray(Wq, np.float32); Wk = np.asarray(Wk, np.float32)
    Wv = np.asarray(Wv, np.float32); Wo = np.asarray(Wo, np.float32)

    if "nc" not in _program_cache:
        _program_cache["nc"] = _build_program()
    nc = _program_cache["nc"]

    perm = _perm()
    # per-batch shared tensors
    xTs, ems = [], []
    for bi in range(B):
        xTs.append(np.ascontiguousarray(hidden_states[bi].T).astype(BF16_NP))
        em = np.zeros((128, 16, 512), np.float32)
        for c in range(4):
            blk = attention_mask[bi, 0, 512 * c:512 * (c + 1), 512 * c:512 * (c + 1)]
            emb = np.exp(blk).T  # [kv, q]
            for dd in range(4):
                em[:, 4 * c + dd, :] = emb[128 * dd:128 * (dd + 1), :]
        ems.append(em.astype(BF16_NP))

    wqTs, wkTs, wvTs, woTs = [], [], [], []
    for j in range(4):
        rows = []
        for m in range(4):
            rows.append(Wq[64 * (8 * j + m):64 * (8 * j + m) + 64])
            rows.append(Wq[64 * (8 * j + 4 + m):64 * (8 * j + 4 + m) + 64])
        wq_core = np.concatenate(rows, 0)
        wqTs.append(np.ascontiguousarray(wq_core.T).astype(BF16_NP))
        wkTs.append(np.ascontiguousarray(Wk[128 * j:128 * (j + 1)].T).astype(BF16_NP))
        wvTs.append(np.ascontiguousarray(Wv[128 * j:128 * (j + 1)].T).astype(BF16_NP))
        wo_core = Wo[512 * j:512 * (j + 1)][:, perm]
        woTs.append(np.ascontiguousarray(wo_core.T).astype(BF16_NP))

    in_maps = []
    for core in range(8):
        bi, j = divmod(core, 4)
        in_maps.append({
            "xT": xTs[bi], "em": ems[bi],
            "wqT": wqTs[j], "wkT": wkTs[j], "wvT": wvTs[j], "woT": woTs[j],
        })

    global _last_results
    res = run_bass_kernel_spmd(nc, in_maps, list(range(8)), **_trace_opts)
    _last_results = res
    out = np.zeros((B, S, H), np.float32)
    for core in range(8):
        bi, j = divmod(core, 4)
        out[bi, :, 512 * j:512 * (j + 1)] = res.results[core]["out_part"]
    return out


if __name__ == "__main__":
    ins = {
        "hidden_states": np.random.randn(B, S, H).astype(np.float32),
        "attention_mask": np.zeros((B, 1, S, S), np.float32),
        "Wq": np.random.randn(2048, H).astype(np.float32) * H ** -0.5,
        "Wk": np.random.randn(512, H).astype(np.float32) * H ** -0.5,
        "Wv": np.random.randn(512, H).astype(np.float32) * H ** -0.5,
        "Wo": np.random.randn(H, 2048).astype(np.float32) * H ** -0.5,
    }
    o = kernel(**ins)
    print("ran", o.shape, o.dtype)
